# revision 9
# baseline (speedup 1.0000x reference)
"""Trainium2 Bass kernel for nn_AttentionBlock (GroupNorm + 8-head self-attention + residual).

Full inputs in, full output out. Sharding: data-parallel over batch across the
8 NeuronCores (16 batches -> 2 per core), weights replicated, no collectives.

Layout strategy (per core, per batch; C=512 channels, S=1024 tokens):
  - x and xhat live as [C, S] tiles (channels on partitions) so GroupNorm
    scale/bias are per-partition scalars.  Cross-partition group reductions
    (16 channels/group) and the broadcast back are tiny PE matmuls against
    one-hot group matrices.
  - Q^T, K^T computed as [128, S] head-pair tiles (partitions = 2 heads x 64
    dk rows); V as [S, 8*65] with a ones-column per head (row-sums of exp
    fall out of the P@V matmul).
  - scores are computed TRANSPOSED: scoresT[j, i] = k_j . q_i so that the
    softmax reduction (over j) aligns with the matmul contraction axis and no
    transposes are ever needed.  The per-head K=64 contraction runs as two
    concurrent row-group matmuls (tile_position rows 0-63 / 64-127), so the
    PE array computes both heads of a pair at once -- no zero padding.
  - exp() runs on ScalarE straight out of PSUM; ScalarE is the kernel's
    critical engine (S*S*heads exps), everything else is scheduled to hide
    under it.
  - P@V gives resU^T [65, S] (row 64 = softmax denominators); normalization
    is reciprocal_approx_fast on the sums row + gpsimd partition_broadcast +
    one DVE mul.
  - out-projection consumes resT directly; residual-add fused in the epilogue
    from the retained x tiles (no DRAM reload).
Attention/projection matmuls run in bf16 (fp32 PSUM accumulation); groupnorm
statistics stay fp32.  The softmax max-subtraction is skipped: scores are
~N(0,1) by construction (standardized activations, 1/sqrt(dk) folded into the
Q weights host-side), so exp() stays comfortably in fp32 range.
"""

import numpy as np
import ml_dtypes

import concourse.bacc as bacc
import concourse.tile as tile
from concourse import mybir
from concourse.bass_utils import run_bass_kernel_spmd

N_CORES = 8
B, C, H, W = 16, 512, 32, 32
S = H * W                      # 1024
BL = B // N_CORES              # 2 batches per core
NH, DK = 8, 64
NG = 32                        # groupnorm groups
GSZ = C // NG                  # 16 channels per group
EPS = 1e-5
F32 = mybir.dt.float32
BF16 = mybir.dt.bfloat16
AF = mybir.ActivationFunctionType
OP = mybir.AluOpType
NPBF16 = ml_dtypes.bfloat16

# test.py can flip these; results stashed in LAST.
TRACE = False
LAST = {}


def _build(has_bqk, has_bv, has_outb):
    nc = bacc.Bacc()

    x_d = nc.dram_tensor("x", [BL, C, S], F32, kind="ExternalInput")
    wqt_d = nc.dram_tensor("wqt", [C, C], BF16, kind="ExternalInput")   # [c_in, q_row]
    wkt_d = nc.dram_tensor("wkt", [C, C], BF16, kind="ExternalInput")
    wvt_d = nc.dram_tensor("wvt", [C, C], BF16, kind="ExternalInput")
    wot_d = nc.dram_tensor("wot", [C, C], BF16, kind="ExternalInput")   # [d_out, c_out]
    g_d = nc.dram_tensor("gmat", [128, 8], F32, kind="ExternalInput")
    gt_d = nc.dram_tensor("gtmat", [8, 128], F32, kind="ExternalInput")
    bqk_d = (
        nc.dram_tensor("bqk", [128, 8], F32, kind="ExternalInput") if has_bqk else None
    )
    bv_d = nc.dram_tensor("bv", [1, C], BF16, kind="ExternalInput") if has_bv else None
    outb_d = (
        nc.dram_tensor("outb", [128, 4], F32, kind="ExternalInput") if has_outb else None
    )
    out_d = nc.dram_tensor("out", [BL, C, S], F32, kind="ExternalOutput")

    with tile.TileContext(nc) as tc:
        with (
            tc.tile_pool(name="const", bufs=1) as const,
            tc.tile_pool(name="px", bufs=8) as px,
            tc.tile_pool(name="pgn", bufs=4) as pgn,
            tc.tile_pool(name="pxh", bufs=8) as pxh,
            tc.tile_pool(name="pqt", bufs=8) as pqt,
            tc.tile_pool(name="pkt", bufs=8) as pkt,
            tc.tile_pool(name="pv", bufs=16) as pvp,
            tc.tile_pool(name="pexp", bufs=20) as pexp,
            tc.tile_pool(name="prec", bufs=4) as prec,
            tc.tile_pool(name="prt", bufs=8) as prt,
            tc.tile_pool(name="pout", bufs=3) as pout,
            tc.tile_pool(name="pps", bufs=2, space="PSUM") as pps,
            tc.tile_pool(name="psc", bufs=2, space="PSUM") as psc,
            tc.tile_pool(name="ppv", bufs=2, space="PSUM") as ppv,
        ):
            # ---- batch-0 x first (groupnorm needs it before weights)
            xt0 = []
            for cb in range(4):
                t = px.tile([128, S], F32, tag="x", name=f"x0_{cb}")
                nc.sync.dma_start(out=t, in_=x_d[0, cb * 128 : (cb + 1) * 128, :])
                xt0.append(t)

            # ---- constants into SBUF (small groupnorm mats first)
            g_sb = const.tile([128, 8], F32, tag="g")
            nc.sync.dma_start(out=g_sb, in_=g_d[:, :])
            gt_sb = const.tile([8, 128], F32, tag="gt")
            nc.sync.dma_start(out=gt_sb, in_=gt_d[:, :])
            wq_sb, wk_sb, wv_sb, wo_sb = [], [], [], []
            for nm, lst, src in (
                ("q", wq_sb, wqt_d),
                ("k", wk_sb, wkt_d),
                ("v", wv_sb, wvt_d),
                ("o", wo_sb, wot_d),
            ):
                for cb in range(4):
                    t = const.tile([128, C], BF16, tag=f"w_{nm}_{cb}")
                    nc.sync.dma_start(out=t, in_=src[cb * 128 : (cb + 1) * 128, :])
                    lst.append(t)
            if has_bqk:
                bqk_sb = const.tile([128, 8], F32, tag="bqk")
                nc.sync.dma_start(out=bqk_sb, in_=bqk_d[:, :])
            if has_bv:
                bv_sb = const.tile([1, C], BF16, tag="bv")
                nc.sync.dma_start(out=bv_sb, in_=bv_d[:, :])
                ones_sb = const.tile([1, S], BF16, tag="ones")
                nc.vector.memset(ones_sb, 1.0)
            if has_outb:
                outb_sb = const.tile([128, 4], F32, tag="outb")
                nc.sync.dma_start(out=outb_sb, in_=outb_d[:, :])

            # ---- PE warm-up: dense dummy matmuls during the DMA/groupnorm
            # startup so the HAM clock gate opens before the first real
            # projection matmul arrives.
            warm_ps = pps.tile([8, 128], F32, tag="pp", name="warm_ps")
            for wi in range(12):
                nc.tensor.matmul(
                    out=warm_ps,
                    lhsT=g_sb,
                    rhs=xt0[0][:, 0:128],
                    start=True,
                    stop=True,
                )

            # ================= emission helpers =================
            def load_x(b):
                xt = []
                for cb in range(4):
                    t = px.tile([128, S], F32, tag="x", name=f"x{b}_{cb}")
                    nc.sync.dma_start(out=t, in_=x_d[b, cb * 128 : (cb + 1) * 128, :])
                    xt.append(t)
                return xt

            def gn_batch(b, xt, xh):
                # groupnorm -> xhat for all 4 channel blocks; rstd computed on
                # DVE only (reciprocal seed + 2 Newton rsqrt steps) so ScalarE
                # never loads a non-Exp activation table.
                pgall = pps.tile([8, 4, 2], F32, tag="pp")   # [group, cb, (mean,e2)]
                for cb in range(4):
                    st6 = pgn.tile([128, 2, 6], F32, tag="st6")
                    nc.vector.bn_stats(out=st6[:, 0, :], in_=xt[cb][:, 0:512])
                    nc.vector.bn_stats(out=st6[:, 1, :], in_=xt[cb][:, 512:1024])
                    mv = pgn.tile([128, 2], F32, tag="mv")
                    nc.vector.bn_aggr(out=mv, in_=st6)
                    me2 = pgn.tile([128, 2], F32, tag="me2")
                    nc.vector.tensor_copy(out=me2[:, 0:1], in_=mv[:, 0:1])
                    nc.vector.tensor_tensor(
                        out=me2[:, 1:2], in0=mv[:, 0:1], in1=mv[:, 0:1], op=OP.mult
                    )
                    nc.vector.tensor_tensor(
                        out=me2[:, 1:2], in0=me2[:, 1:2], in1=mv[:, 1:2], op=OP.add
                    )
                    nc.tensor.matmul(
                        out=pgall[:, cb, :], lhsT=g_sb, rhs=me2, start=True, stop=True
                    )
                # group stats for all blocks at once ([8, 4] tiles)
                gm = pgn.tile([8, 4], F32, tag="gm")
                z = pgn.tile([8, 4], F32, tag="z")
                t2 = pgn.tile([8, 4], F32, tag="t2")
                y = pgn.tile([8, 4], F32, tag="y")
                nc.vector.tensor_scalar(
                    out=gm, in0=pgall[:, :, 0], scalar1=1.0 / GSZ, scalar2=None,
                    op0=OP.mult,
                )
                nc.vector.tensor_scalar(
                    out=z, in0=pgall[:, :, 1], scalar1=1.0 / GSZ, scalar2=EPS,
                    op0=OP.mult, op1=OP.add,
                )
                nc.vector.tensor_tensor(out=t2, in0=gm, in1=gm, op=OP.mult)
                nc.vector.tensor_tensor(out=z, in0=z, in1=t2, op=OP.subtract)
                # rsqrt(z): y0 = 1/z, then y <- y*(1.5 - 0.5*z*y^2) twice
                nc.vector.reciprocal(out=y, in_=z)
                for _ in range(2):
                    nc.vector.tensor_tensor(out=t2, in0=z, in1=y, op=OP.mult)
                    nc.vector.tensor_tensor(out=t2, in0=t2, in1=y, op=OP.mult)
                    nc.vector.tensor_scalar(
                        out=t2, in0=t2, scalar1=-0.5, scalar2=1.5,
                        op0=OP.mult, op1=OP.add,
                    )
                    nc.vector.tensor_tensor(out=y, in0=y, in1=t2, op=OP.mult)
                gs2 = pgn.tile([8, 2, 4], F32, tag="gs2")   # [(mean,rstd), cb]
                nc.vector.tensor_copy(out=gs2[:, 0, :], in_=gm)
                nc.vector.tensor_copy(out=gs2[:, 1, :], in_=y)
                for cb in range(4):
                    pb = pps.tile([128, 2], F32, tag="pp")
                    nc.tensor.matmul(
                        out=pb, lhsT=gt_sb, rhs=gs2[:, :, cb], start=True, stop=True
                    )
                    t = pxh.tile([128, S], BF16, tag="xh", name=f"xh{b}_{cb}")
                    nc.vector.tensor_scalar(
                        out=t,
                        in0=xt[cb],
                        scalar1=pb[:, 0:1],
                        scalar2=pb[:, 1:2],
                        op0=OP.subtract,
                        op1=OP.mult,
                    )
                    xh.append(t)

            def v_group(b, xh, vt, st):
                # one [S-tile, NH, 65] V tile with ones column per head
                pv = pps.tile([128, 512], F32, tag="pp")
                for cb in range(4):
                    nc.tensor.matmul(
                        out=pv,
                        lhsT=xh[cb][:, st * 128 : (st + 1) * 128],
                        rhs=wv_sb[cb],
                        start=(cb == 0),
                        stop=(cb == 3 and not has_bv),
                    )
                if has_bv:
                    nc.tensor.matmul(
                        out=pv,
                        lhsT=ones_sb[:, st * 128 : (st + 1) * 128],
                        rhs=bv_sb,
                        start=False,
                        stop=True,
                    )
                t = pvp.tile([128, NH, 65], BF16, tag="v", name=f"v{b}_{st}")
                nc.vector.memset(t[:, :, 64:65], 1.0)
                nc.vector.tensor_copy(
                    out=t[:, :, 0:64], in_=pv.rearrange("p (h d) -> p h d", h=NH)
                )
                vt.append(t)

            def qk_units(b, xh, dst, w_sb, boff, rb):
                # one projection psum row-block -> one [128, S] head-pair tile
                # (two 512-column half-units so fill slots stay small).
                holder = {}
                pool = pqt if boff == 0 else pkt
                pfx = "q" if boff == 0 else "k"

                def half(sc):
                    if "t" not in holder:
                        t = pool.tile([128, S], BF16, tag="qk", name=f"{pfx}{b}_{rb}")
                        holder["t"] = t
                        dst.append(t)
                    t = holder["t"]
                    pq = pps.tile([128, 512], F32, tag="pp")
                    for cb in range(4):
                        nc.tensor.matmul(
                            out=pq,
                            lhsT=w_sb[cb][:, rb * 128 : (rb + 1) * 128],
                            rhs=xh[cb][:, sc * 512 : (sc + 1) * 512],
                            start=(cb == 0),
                            stop=(cb == 3),
                        )
                    cols = slice(sc * 512, (sc + 1) * 512)
                    if has_bqk:
                        nc.vector.tensor_scalar_add(
                            out=t[:, cols],
                            in0=pq,
                            scalar1=bqk_sb[:, boff + rb : boff + rb + 1],
                        )
                    else:
                        nc.vector.tensor_copy(out=t[:, cols], in_=pq)

                return [lambda: half(0), lambda: half(1)]

            def epi_units(b, xt, rt, cb):
                # epi_block split into two 512-column half-units (DMA on 2nd);
                # residual comes from the retained x tiles.
                holder = {}

                def half(sc):
                    if "t" not in holder:
                        holder["t"] = pout.tile(
                            [128, S], F32, tag="ot", name=f"ot{b}_{cb}"
                        )
                    ot = holder["t"]
                    po = pps.tile([128, 512], F32, tag="pp")
                    for db in range(4):
                        nc.tensor.matmul(
                            out=po,
                            lhsT=wo_sb[db][:, cb * 128 : (cb + 1) * 128],
                            rhs=rt[db][:, sc * 512 : (sc + 1) * 512],
                            start=(db == 0),
                            stop=(db == 3),
                        )
                    cols = slice(sc * 512, (sc + 1) * 512)
                    dst_ap = ot[:, cols]
                    if has_outb:
                        nc.vector.scalar_tensor_tensor(
                            out=dst_ap,
                            in0=po,
                            scalar=outb_sb[:, cb : cb + 1],
                            in1=xt[cb][:, cols],
                            op0=OP.add,
                            op1=OP.add,
                        )
                    else:
                        nc.vector.tensor_tensor(
                            out=dst_ap, in0=po, in1=xt[cb][:, cols], op=OP.add
                        )
                    if sc == 1:
                        nc.sync.dma_start(
                            out=out_d[b, cb * 128 : (cb + 1) * 128, :], in_=ot
                        )

                return [lambda: half(0), lambda: half(1)]

            queue = []

            def fill(n=1):
                for _ in range(min(n, len(queue))):
                    queue.pop(0)()

            def attn_phaseA(b, qt, kt, hp, fills_per_jb=2):
                # transposed scores for both heads of the pair via concurrent
                # row-group matmuls (K=64 each), exp per head/jb.  Returns the
                # exp tiles; P@V/normalize are queued separately (pb_units) so
                # they interleave with the NEXT pair's scores and ScalarE
                # never starves.
                ex = [[None] * 8, [None] * 8]
                for jb in range(8):
                    # emit the 4 score matmuls alternating row groups
                    # (head-even rows 0-63, head-odd rows 64-127) so each
                    # LDWEIGHTS pulls ahead of the other group's in-flight
                    # matmul and the pair computes concurrently on the PE.
                    pss = [
                        psc.tile([128, S], F32, tag="ps", name=f"ps{hi}")
                        for hi in range(2)
                    ]
                    for sc in range(2):
                        for hi in range(2):
                            prng = slice(hi * 64, (hi + 1) * 64)
                            cols = slice(sc * 512, (sc + 1) * 512)
                            nc.tensor.matmul(
                                out=pss[hi][:, cols],
                                lhsT=kt[hp][prng, jb * 128 : (jb + 1) * 128],
                                rhs=qt[hp][prng, cols],
                                start=True,
                                stop=True,
                            )
                    for hi in range(2):
                        e = pexp.tile([128, S], BF16, tag="ex")
                        nc.scalar.activation(out=e, in_=pss[hi], func=AF.Exp)
                        ex[hi][jb] = e
                    fill(fills_per_jb)
                return ex

            def pb_units(b, vt, rt, hp, ex):
                # P@V + normalize for one head pair as 6 fill units
                units = []
                for hi in range(2):
                    h = 2 * hp + hi
                    holder = {}

                    def pv_half(lo, hi_=None, hh=None, hld=None):
                        if "p" not in hld:
                            hld["p"] = [
                                ppv.tile([65, 512], F32, tag="ppvt", name=f"pvt{i}")
                                for i in range(2)
                            ]
                        pvts = hld["p"]
                        for jb in range(lo, lo + 4):
                            for sc in range(2):
                                nc.tensor.matmul(
                                    out=pvts[sc],
                                    lhsT=vt[jb][:, hh, :],
                                    rhs=ex[hi_][jb][:, sc * 512 : (sc + 1) * 512],
                                    start=(jb == 0),
                                    stop=(jb == 7),
                                )

                    def norm(hi_=None, hld=None):
                        for sc in range(2):
                            pvt = hld["p"][sc]
                            # stage sums row to SBUF: the custom-DVE recip
                            # reads garbage from PSUM on hardware
                            stage = prec.tile([1, 512], F32, tag="st")
                            nc.vector.tensor_copy(out=stage, in_=pvt[64:65, :])
                            rrow = prec.tile([1, 512], F32, tag="rr")
                            nc.vector.reciprocal_approx_fast(out=rrow, in_=stage)
                            rbt = prec.tile([64, 512], F32, tag="rb")
                            nc.gpsimd.partition_broadcast(rbt, rrow)
                            nc.vector.tensor_tensor(
                                out=rt[hp][
                                    hi_ * 64 : (hi_ + 1) * 64,
                                    sc * 512 : (sc + 1) * 512,
                                ],
                                in0=pvt[0:64, :],
                                in1=rbt,
                                op=OP.mult,
                            )

                    units.append(
                        lambda lo=0, hi_=hi, hh=h, hld=holder: pv_half(
                            lo, hi_=hi_, hh=hh, hld=hld
                        )
                    )
                    units.append(
                        lambda lo=4, hi_=hi, hh=h, hld=holder: pv_half(
                            lo, hi_=hi_, hh=hh, hld=hld
                        )
                    )
                    units.append(lambda hi_=hi, hld=holder: norm(hi_=hi_, hld=hld))
                return units

            # ================= schedule =================
            # batch-0 prep emitted directly; everything else (V tiles,
            # remaining projections, batch-1 groupnorm, P@V+normalize of the
            # previous pair, epilogues) is drained from one global work queue
            # two units per jb inside the attention loops, so ScalarE streams
            # exps continuously while the other engines chew through the
            # queue.  P@V units for pair p are PREPENDED when pair p+1 starts
            # so they run early (their exp tiles free up the pexp ring).
            xt1 = load_x(1)
            xh0, qt0, kt0, vt0 = [], [], [], []
            gn_batch(0, xt0, xh0)
            # pair-0/1 row-blocks of Q/K emitted directly: each pair's Q/K
            # must be materialized by the time its phase A is emitted.
            for rb in range(2):
                for u in qk_units(0, xh0, qt0, wq_sb, 0, rb):
                    u()
                for u in qk_units(0, xh0, kt0, wk_sb, 4, rb):
                    u()

            xh1, qt1, kt1, vt1 = [], [], [], []
            for st in range(8):
                queue.append(lambda st=st: v_group(0, xh0, vt0, st))
            for rb in range(2, 4):
                queue.extend(qk_units(0, xh0, qt0, wq_sb, 0, rb))
                queue.extend(qk_units(0, xh0, kt0, wk_sb, 4, rb))
            # batch-1 groupnorm deferred into the queue: its PSUM tiles must
            # sit BEHIND batch-0's V/QK in the pps ring, else the in-order PE
            # head-of-line blocks on xt1's DMA through the pool WAR edge.
            queue.append(lambda: gn_batch(1, xt1, xh1))
            for rb in range(2):
                queue.extend(qk_units(1, xh1, qt1, wq_sb, 0, rb))
                queue.extend(qk_units(1, xh1, kt1, wk_sb, 4, rb))
            for st in range(8):
                queue.append(lambda st=st: v_group(1, xh1, vt1, st))
            for rb in range(2, 4):
                queue.extend(qk_units(1, xh1, qt1, wq_sb, 0, rb))
                queue.extend(qk_units(1, xh1, kt1, wk_sb, 4, rb))

            rt0 = [prt.tile([128, S], BF16, tag="rt", name=f"rt0_{i}") for i in range(4)]
            rt1 = [prt.tile([128, S], BF16, tag="rt", name=f"rt1_{i}") for i in range(4)]
            for hp in range(4):
                ex = attn_phaseA(0, qt0, kt0, hp)
                queue[:0] = pb_units(0, vt0, rt0, hp, ex)
            for cb in range(4):
                queue.extend(epi_units(0, xt0, rt0, cb))
            for hp in range(4):
                ex = attn_phaseA(1, qt1, kt1, hp)
                queue[:0] = pb_units(1, vt1, rt1, hp, ex)
            fill(len(queue))
            for cb in range(4):
                for u in epi_units(1, xt1, rt1, cb):
                    u()

    nc.finalize()
    return nc


def kernel(**inputs):
    x = np.asarray(inputs["x"], np.float32)
    norm_w = np.asarray(inputs["norm_w"], np.float64)
    norm_b = np.asarray(inputs["norm_b"], np.float64)
    proj_w = np.asarray(inputs["proj_w"], np.float64)
    proj_b = np.asarray(inputs["proj_b"], np.float64)
    out_w = np.asarray(inputs["out_w"], np.float32)
    out_b = np.asarray(inputs["out_b"], np.float32)

    # split qkv rows (row = h*192 + t*64 + d, t in {q,k,v}) into head-major mats
    pw = proj_w.reshape(NH, 3, DK, C)
    pb = proj_b.reshape(NH, 3, DK)
    mats, biases = [], []
    for t in range(3):
        wm = pw[:, t].reshape(NH * DK, C)
        bv = pb[:, t].reshape(NH * DK)
        # fold groupnorm affine: y = xhat*nw + nb  =>  W@y + b = (W*nw)@xhat + (W@nb + b)
        mats.append(wm * norm_w[None, :])
        biases.append(bv + wm @ norm_b)
    wq, wk, wv = mats
    bq, bk, bv = biases
    scale = DK ** -0.5
    wq = wq * scale
    bq = bq * scale

    wqT = np.ascontiguousarray(wq.T).astype(NPBF16)
    wkT = np.ascontiguousarray(wk.T).astype(NPBF16)
    wvT = np.ascontiguousarray(wv.T).astype(NPBF16)
    woT = np.ascontiguousarray(out_w.T).astype(NPBF16)

    G = np.zeros((128, 8), np.float32)
    G[np.arange(128), np.arange(128) // GSZ] = 1.0
    GT = np.ascontiguousarray(G.T)

    has_bqk = bool(np.any(bq) or np.any(bk))
    has_bv = bool(np.any(bv))
    has_outb = bool(np.any(out_b))

    bqk = np.zeros((128, 8), np.float32)
    bqk[:, 0:4] = bq.reshape(4, 128).T
    bqk[:, 4:8] = bk.reshape(4, 128).T
    outb128 = np.ascontiguousarray(out_b.reshape(4, 128).T)

    nc = _build(has_bqk, has_bv, has_outb)

    xr = x.reshape(B, C, S)
    in_maps = []
    for c in range(N_CORES):
        m = {
            "x": np.ascontiguousarray(xr[c * BL : (c + 1) * BL]),
            "wqt": wqT,
            "wkt": wkT,
            "wvt": wvT,
            "wot": woT,
            "gmat": G,
            "gtmat": GT,
        }
        if has_bqk:
            m["bqk"] = bqk
        if has_bv:
            m["bv"] = np.ascontiguousarray(bv.reshape(1, C)).astype(NPBF16)
        if has_outb:
            m["outb"] = outb128
        in_maps.append(m)

    # guard: bass_utils imports antenv.axon_hooks when tracing is requested
    # (e.g. via BASS_TRACE env); provide a no-op module if the image lacks it.
    try:
        import antenv.axon_hooks  # noqa: F401
    except ImportError:
        import sys
        import types

        import antenv

        _m = types.ModuleType("antenv.axon_hooks")
        _m._hook = None
        _m.set_axon_ntff_profile_hook = lambda h: setattr(_m, "_hook", h)
        _m.get_axon_ntff_profile_hook = lambda: _m._hook
        sys.modules["antenv.axon_hooks"] = _m
        antenv.axon_hooks = _m

    res = None
    for attempt in range(3):
        try:
            res = run_bass_kernel_spmd(
                nc, in_maps, core_ids=list(range(N_CORES)), trace=TRACE
            )
            break
        except Exception:
            # transient NRT_EXEC_UNIT_UNRECOVERABLE-style device hiccups
            # clear on retry; re-raise on the final attempt
            if attempt == 2:
                raise
    LAST["exec_time_ns"] = res.exec_time_ns
    LAST["mean_exec_time_ns"] = res.mean_exec_time_ns
    LAST["result"] = res

    out = np.concatenate([res.results[c]["out"] for c in range(N_CORES)], axis=0)
    return np.ascontiguousarray(out.reshape(B, C, H, W).astype(np.float32))


# revision 17
# speedup vs baseline: 1.0524x; 1.0524x over previous
"""Trainium2 Bass kernel for nn_AttentionBlock (GroupNorm + 8-head self-attention + residual).

Full inputs in, full output out. Sharding: data-parallel over batch across the
8 NeuronCores (16 batches -> 2 per core), weights replicated, no collectives.

Layout strategy (per core, per batch; C=512 channels, S=1024 tokens):
  - x and xhat live as [C, S] tiles (channels on partitions) so GroupNorm
    scale/bias are per-partition scalars.  Cross-partition group reductions
    (16 channels/group) and the broadcast back are tiny PE matmuls against
    one-hot group matrices.
  - Q^T, K^T computed as [128, S] head-pair tiles (partitions = 2 heads x 64
    dk rows); V as [S, 8*65] with a ones-column per head (row-sums of exp
    fall out of the P@V matmul).
  - scores are computed TRANSPOSED: scoresT[j, i] = k_j . q_i so that the
    softmax reduction (over j) aligns with the matmul contraction axis and no
    transposes are ever needed.  The per-head K=64 contraction runs as two
    concurrent row-group matmuls (tile_position rows 0-63 / 64-127), so the
    PE array computes both heads of a pair at once -- no zero padding.
  - exp() runs on ScalarE straight out of PSUM; ScalarE is the kernel's
    critical engine (S*S*heads exps), everything else is scheduled to hide
    under it.
  - P@V gives resU^T [65, S] (row 64 = softmax denominators); normalization
    is reciprocal_approx_fast on the sums row + gpsimd partition_broadcast +
    one DVE mul.
  - out-projection consumes resT directly; residual-add fused in the epilogue
    from the retained x tiles (no DRAM reload).
Attention/projection matmuls run in bf16 (fp32 PSUM accumulation); groupnorm
statistics stay fp32.  The softmax max-subtraction is skipped: scores are
~N(0,1) by construction (standardized activations, 1/sqrt(dk) folded into the
Q weights host-side), so exp() stays comfortably in fp32 range.
"""

import numpy as np
import ml_dtypes

import concourse.bacc as bacc
import concourse.tile as tile
from concourse import mybir
from concourse.bass_utils import run_bass_kernel_spmd

N_CORES = 8
B, C, H, W = 16, 512, 32, 32
S = H * W                      # 1024
BL = B // N_CORES              # 2 batches per core
NH, DK = 8, 64
NG = 32                        # groupnorm groups
GSZ = C // NG                  # 16 channels per group
EPS = 1e-5
F32 = mybir.dt.float32
BF16 = mybir.dt.bfloat16
FP8 = mybir.dt.float8e4
DR = mybir.MatmulPerfMode.DoubleRow
AF = mybir.ActivationFunctionType
OP = mybir.AluOpType
NPBF16 = ml_dtypes.bfloat16
# exp(s - EXP_SHIFT) keeps P inside e4m3 range (max ~240); the shift cancels
# exactly in the softmax normalization.
EXP_SHIFT = 1.5

# test.py can flip these; results stashed in LAST.
TRACE = False
LAST = {}


def _build(has_bqk, has_bv, has_outb):
    nc = bacc.Bacc()

    x_d = nc.dram_tensor("x", [BL, C, S], F32, kind="ExternalInput")
    wqt_d = nc.dram_tensor("wqt", [C, C], BF16, kind="ExternalInput")   # [c_in, q_row]
    wkt_d = nc.dram_tensor("wkt", [C, C], BF16, kind="ExternalInput")
    wvt_d = nc.dram_tensor("wvt", [C, C], BF16, kind="ExternalInput")
    wot_d = nc.dram_tensor("wot", [C, C], BF16, kind="ExternalInput")   # [d_out, c_out]
    g_d = nc.dram_tensor("gmat", [128, 8], F32, kind="ExternalInput")
    gt_d = nc.dram_tensor("gtmat", [8, 128], F32, kind="ExternalInput")
    bqk_d = (
        nc.dram_tensor("bqk", [128, 8], F32, kind="ExternalInput") if has_bqk else None
    )
    bv_d = nc.dram_tensor("bv", [1, C], BF16, kind="ExternalInput") if has_bv else None
    outb_d = (
        nc.dram_tensor("outb", [128, 4], F32, kind="ExternalInput") if has_outb else None
    )
    out_d = nc.dram_tensor("out", [BL, C, S], F32, kind="ExternalOutput")

    with tile.TileContext(nc) as tc:
        with (
            tc.tile_pool(name="const", bufs=1) as const,
            tc.tile_pool(name="px", bufs=8) as px,
            tc.tile_pool(name="pgn", bufs=4) as pgn,
            tc.tile_pool(name="pxh", bufs=8) as pxh,
            tc.tile_pool(name="pqt", bufs=8) as pqt,
            tc.tile_pool(name="pkt", bufs=8) as pkt,
            tc.tile_pool(name="pv", bufs=10) as pvp,
            tc.tile_pool(name="pexp", bufs=20) as pexp,
            tc.tile_pool(name="prec", bufs=4) as prec,
            tc.tile_pool(name="prt", bufs=8) as prt,
            tc.tile_pool(name="pout", bufs=3) as pout,
            tc.tile_pool(name="pps", bufs=2, space="PSUM") as pps,
            tc.tile_pool(name="psc", bufs=2, space="PSUM") as psc,
            tc.tile_pool(name="ppv", bufs=2, space="PSUM") as ppv,
        ):
            # ---- batch-0 x first (groupnorm needs it before weights)
            xt0 = []
            for cb in range(4):
                t = px.tile([128, S], F32, tag="x", name=f"x0_{cb}")
                nc.sync.dma_start(out=t, in_=x_d[0, cb * 128 : (cb + 1) * 128, :])
                xt0.append(t)

            # ---- constants into SBUF (small groupnorm mats first)
            g_sb = const.tile([128, 8], F32, tag="g")
            nc.sync.dma_start(out=g_sb, in_=g_d[:, :])
            gt_sb = const.tile([8, 128], F32, tag="gt")
            nc.sync.dma_start(out=gt_sb, in_=gt_d[:, :])
            wq_sb, wk_sb, wv_sb, wo_sb = [], [], [], []
            for nm, lst, src in (
                ("q", wq_sb, wqt_d),
                ("k", wk_sb, wkt_d),
                ("v", wv_sb, wvt_d),
                ("o", wo_sb, wot_d),
            ):
                for cb in range(4):
                    t = const.tile([128, C], BF16, tag=f"w_{nm}_{cb}")
                    nc.sync.dma_start(out=t, in_=src[cb * 128 : (cb + 1) * 128, :])
                    lst.append(t)
            shift_sb = const.tile([128, 1], F32, tag="shift")
            nc.vector.memset(shift_sb, -EXP_SHIFT)
            if has_bqk:
                bqk_sb = const.tile([128, 8], F32, tag="bqk")
                nc.sync.dma_start(out=bqk_sb, in_=bqk_d[:, :])
            if has_bv:
                bv_sb = const.tile([1, C], BF16, tag="bv")
                nc.sync.dma_start(out=bv_sb, in_=bv_d[:, :])
                ones_sb = const.tile([1, S], BF16, tag="ones")
                nc.vector.memset(ones_sb, 1.0)
            if has_outb:
                outb_sb = const.tile([128, 4], F32, tag="outb")
                nc.sync.dma_start(out=outb_sb, in_=outb_d[:, :])

            # ---- PE warm-up: dense dummy matmuls during the DMA/groupnorm
            # startup so the HAM clock gate opens before the first real
            # projection matmul arrives.
            warm_ps = pps.tile([8, 128], F32, tag="pp", name="warm_ps")
            for wi in range(12):
                nc.tensor.matmul(
                    out=warm_ps,
                    lhsT=g_sb,
                    rhs=xt0[0][:, 0:128],
                    start=True,
                    stop=True,
                )

            # ================= emission helpers =================
            def load_x(b):
                xt = []
                for cb in range(4):
                    t = px.tile([128, S], F32, tag="x", name=f"x{b}_{cb}")
                    nc.sync.dma_start(out=t, in_=x_d[b, cb * 128 : (cb + 1) * 128, :])
                    xt.append(t)
                return xt

            def gn_batch(b, xt, xh):
                # groupnorm -> xhat for all 4 channel blocks; rstd computed on
                # DVE only (reciprocal seed + 2 Newton rsqrt steps) so ScalarE
                # never loads a non-Exp activation table.
                pgall = pps.tile([8, 4, 2], F32, tag="pp")   # [group, cb, (mean,e2)]
                for cb in range(4):
                    st6 = pgn.tile([128, 2, 6], F32, tag="st6")
                    nc.vector.bn_stats(out=st6[:, 0, :], in_=xt[cb][:, 0:512])
                    nc.vector.bn_stats(out=st6[:, 1, :], in_=xt[cb][:, 512:1024])
                    mv = pgn.tile([128, 2], F32, tag="mv")
                    nc.vector.bn_aggr(out=mv, in_=st6)
                    me2 = pgn.tile([128, 2], F32, tag="me2")
                    nc.vector.tensor_copy(out=me2[:, 0:1], in_=mv[:, 0:1])
                    nc.vector.tensor_tensor(
                        out=me2[:, 1:2], in0=mv[:, 0:1], in1=mv[:, 0:1], op=OP.mult
                    )
                    nc.vector.tensor_tensor(
                        out=me2[:, 1:2], in0=me2[:, 1:2], in1=mv[:, 1:2], op=OP.add
                    )
                    nc.tensor.matmul(
                        out=pgall[:, cb, :], lhsT=g_sb, rhs=me2, start=True, stop=True
                    )
                # group stats for all blocks at once ([8, 4] tiles)
                gm = pgn.tile([8, 4], F32, tag="gm")
                z = pgn.tile([8, 4], F32, tag="z")
                t2 = pgn.tile([8, 4], F32, tag="t2")
                y = pgn.tile([8, 4], F32, tag="y")
                nc.vector.tensor_scalar(
                    out=gm, in0=pgall[:, :, 0], scalar1=1.0 / GSZ, scalar2=None,
                    op0=OP.mult,
                )
                nc.vector.tensor_scalar(
                    out=z, in0=pgall[:, :, 1], scalar1=1.0 / GSZ, scalar2=EPS,
                    op0=OP.mult, op1=OP.add,
                )
                nc.vector.tensor_tensor(out=t2, in0=gm, in1=gm, op=OP.mult)
                nc.vector.tensor_tensor(out=z, in0=z, in1=t2, op=OP.subtract)
                # rsqrt(z): y0 = 1/z, then y <- y*(1.5 - 0.5*z*y^2) twice
                nc.vector.reciprocal(out=y, in_=z)
                for _ in range(2):
                    nc.vector.tensor_tensor(out=t2, in0=z, in1=y, op=OP.mult)
                    nc.vector.tensor_tensor(out=t2, in0=t2, in1=y, op=OP.mult)
                    nc.vector.tensor_scalar(
                        out=t2, in0=t2, scalar1=-0.5, scalar2=1.5,
                        op0=OP.mult, op1=OP.add,
                    )
                    nc.vector.tensor_tensor(out=y, in0=y, in1=t2, op=OP.mult)
                gs2 = pgn.tile([8, 2, 4], F32, tag="gs2")   # [(mean,rstd), cb]
                nc.vector.tensor_copy(out=gs2[:, 0, :], in_=gm)
                nc.vector.tensor_copy(out=gs2[:, 1, :], in_=y)
                for cb in range(4):
                    pb = pps.tile([128, 2], F32, tag="pp")
                    nc.tensor.matmul(
                        out=pb, lhsT=gt_sb, rhs=gs2[:, :, cb], start=True, stop=True
                    )
                    t = pxh.tile([128, S], BF16, tag="xh", name=f"xh{b}_{cb}")
                    nc.vector.tensor_scalar(
                        out=t,
                        in0=xt[cb],
                        scalar1=pb[:, 0:1],
                        scalar2=pb[:, 1:2],
                        op0=OP.subtract,
                        op1=OP.mult,
                    )
                    xh.append(t)

            def v_group(b, xh, vt, st):
                # V rows for one 128-token S-tile, written into the jb-PAIR
                # fp8 tile [128, 2, NH, 72] (pair dim q = st%2 feeds the
                # DoubleRow P@V contraction; 72-col head stride keeps the
                # pair-dim byte step 16-aligned; col 64 is the ones column).
                pv = pps.tile([128, 512], F32, tag="pp")
                for cb in range(4):
                    nc.tensor.matmul(
                        out=pv,
                        lhsT=xh[cb][:, st * 128 : (st + 1) * 128],
                        rhs=wv_sb[cb],
                        start=(cb == 0),
                        stop=(cb == 3 and not has_bv),
                    )
                if has_bv:
                    nc.tensor.matmul(
                        out=pv,
                        lhsT=ones_sb[:, st * 128 : (st + 1) * 128],
                        rhs=bv_sb,
                        start=False,
                        stop=True,
                    )
                if st % 2 == 0:
                    t = pvp.tile([128, 2, NH, 72], FP8, tag="v", name=f"v{b}_{st}")
                    vt.append(t)
                t = vt[st // 2]
                q = st % 2
                nc.vector.memset(t[:, q, :, 64:65], 1.0)
                nc.vector.tensor_copy(
                    out=t[:, q, :, 0:64], in_=pv.rearrange("p (h d) -> p h d", h=NH)
                )

            def qk_units(b, xh, dst, w_sb, boff, rb):
                # one projection psum row-block -> one [128, S] head-pair tile
                # (two 512-column half-units so fill slots stay small).
                holder = {}
                pool = pqt if boff == 0 else pkt
                pfx = "q" if boff == 0 else "k"

                def half(sc):
                    if "t" not in holder:
                        t = pool.tile([128, S], BF16, tag="qk", name=f"{pfx}{b}_{rb}")
                        holder["t"] = t
                        dst.append(t)
                    t = holder["t"]
                    pq = pps.tile([128, 512], F32, tag="pp")
                    for cb in range(4):
                        nc.tensor.matmul(
                            out=pq,
                            lhsT=w_sb[cb][:, rb * 128 : (rb + 1) * 128],
                            rhs=xh[cb][:, sc * 512 : (sc + 1) * 512],
                            start=(cb == 0),
                            stop=(cb == 3),
                        )
                    cols = slice(sc * 512, (sc + 1) * 512)
                    if has_bqk:
                        nc.vector.tensor_scalar_add(
                            out=t[:, cols],
                            in0=pq,
                            scalar1=bqk_sb[:, boff + rb : boff + rb + 1],
                        )
                    else:
                        nc.vector.tensor_copy(out=t[:, cols], in_=pq)

                return [lambda: half(0), lambda: half(1)]

            def epi_units(b, xt, rt, cb):
                # epi_block split into two 512-column half-units (DMA on 2nd);
                # residual comes from the retained x tiles.
                holder = {}

                def half(sc):
                    if "t" not in holder:
                        holder["t"] = pout.tile(
                            [128, S], F32, tag="ot", name=f"ot{b}_{cb}"
                        )
                    ot = holder["t"]
                    po = pps.tile([128, 512], F32, tag="pp")
                    for db in range(4):
                        nc.tensor.matmul(
                            out=po,
                            lhsT=wo_sb[db][:, cb * 128 : (cb + 1) * 128],
                            rhs=rt[db][:, sc * 512 : (sc + 1) * 512],
                            start=(db == 0),
                            stop=(db == 3),
                        )
                    cols = slice(sc * 512, (sc + 1) * 512)
                    dst_ap = ot[:, cols]
                    if has_outb:
                        nc.vector.scalar_tensor_tensor(
                            out=dst_ap,
                            in0=po,
                            scalar=outb_sb[:, cb : cb + 1],
                            in1=xt[cb][:, cols],
                            op0=OP.add,
                            op1=OP.add,
                        )
                    else:
                        nc.vector.tensor_tensor(
                            out=dst_ap, in0=po, in1=xt[cb][:, cols], op=OP.add
                        )
                    if sc == 1:
                        nc.sync.dma_start(
                            out=out_d[b, cb * 128 : (cb + 1) * 128, :], in_=ot
                        )

                return [lambda: half(0), lambda: half(1)]

            queue = []

            def fill(n=1):
                for _ in range(min(n, len(queue))):
                    queue.pop(0)()

            def attn_phaseA(b, qt, kt, hp, fills_per_jb=2):
                # transposed scores for both heads of the pair via concurrent
                # row-group matmuls (K=64 each), exp per head/jb.  Returns the
                # exp tiles; P@V/normalize are queued separately (pb_units) so
                # they interleave with the NEXT pair's scores and ScalarE
                # never starves.
                ex = [[None] * 4, [None] * 4]
                for jb in range(8):
                    jp, q = jb // 2, jb % 2
                    if q == 0:
                        for hi in range(2):
                            e = pexp.tile(
                                [128, 2, S], FP8, tag="ex", name=f"ex{hi}"
                            )
                            ex[hi][jp] = e
                    # emit the 4 score matmuls alternating row groups
                    # (head-even rows 0-63, head-odd rows 64-127) so each
                    # LDWEIGHTS pulls ahead of the other group's in-flight
                    # matmul and the pair computes concurrently on the PE.
                    pss = [
                        psc.tile([128, S], F32, tag="ps", name=f"ps{hi}")
                        for hi in range(2)
                    ]
                    for sc in range(2):
                        for hi in range(2):
                            prng = slice(hi * 64, (hi + 1) * 64)
                            cols = slice(sc * 512, (sc + 1) * 512)
                            nc.tensor.matmul(
                                out=pss[hi][:, cols],
                                lhsT=kt[hp][prng, jb * 128 : (jb + 1) * 128],
                                rhs=qt[hp][prng, cols],
                                start=True,
                                stop=True,
                            )
                    for hi in range(2):
                        nc.scalar.activation(
                            out=ex[hi][jp][:, q, :], in_=pss[hi], func=AF.Exp,
                            bias=shift_sb[:, 0:1],
                        )
                    fill(fills_per_jb)
                return ex

            def pb_units(b, vt, rt, hp, ex):
                # P@V + normalize for one head pair as 6 fill units
                units = []
                for hi in range(2):
                    h = 2 * hp + hi
                    holder = {}

                    def pv_half(lo, hi_=None, hh=None, hld=None):
                        # DoubleRow P@V: each matmul contracts a 256-token
                        # jb-pair (2 fp8 rows per PE cell)
                        if "p" not in hld:
                            hld["p"] = [
                                ppv.tile([65, 512], F32, tag="ppvt", name=f"pvt{i}")
                                for i in range(2)
                            ]
                        pvts = hld["p"]
                        for jp in range(lo, lo + 2):
                            for sc in range(2):
                                nc.tensor.matmul(
                                    out=pvts[sc],
                                    lhsT=vt[jp][:, :, hh, 0:65],
                                    rhs=ex[hi_][jp][:, :, sc * 512 : (sc + 1) * 512],
                                    start=(jp == 0),
                                    stop=(jp == 3),
                                    perf_mode=DR,
                                )

                    def norm(hi_=None, hld=None):
                        for sc in range(2):
                            pvt = hld["p"][sc]
                            # stage sums row to SBUF: the custom-DVE recip
                            # reads garbage from PSUM on hardware
                            stage = prec.tile([1, 512], F32, tag="st")
                            nc.vector.tensor_copy(out=stage, in_=pvt[64:65, :])
                            rrow = prec.tile([1, 512], F32, tag="rr")
                            nc.vector.reciprocal_approx_fast(out=rrow, in_=stage)
                            rbt = prec.tile([64, 512], F32, tag="rb")
                            nc.gpsimd.partition_broadcast(rbt, rrow)
                            nc.vector.tensor_tensor(
                                out=rt[hp][
                                    hi_ * 64 : (hi_ + 1) * 64,
                                    sc * 512 : (sc + 1) * 512,
                                ],
                                in0=pvt[0:64, :],
                                in1=rbt,
                                op=OP.mult,
                            )

                    units.append(
                        lambda lo=0, hi_=hi, hh=h, hld=holder: pv_half(
                            lo, hi_=hi_, hh=hh, hld=hld
                        )
                    )
                    units.append(
                        lambda lo=2, hi_=hi, hh=h, hld=holder: pv_half(
                            lo, hi_=hi_, hh=hh, hld=hld
                        )
                    )
                    units.append(lambda hi_=hi, hld=holder: norm(hi_=hi_, hld=hld))
                return units

            # ================= schedule =================
            # batch-0 prep emitted directly; everything else (V tiles,
            # remaining projections, batch-1 groupnorm, P@V+normalize of the
            # previous pair, epilogues) is drained from one global work queue
            # two units per jb inside the attention loops, so ScalarE streams
            # exps continuously while the other engines chew through the
            # queue.  P@V units for pair p are PREPENDED when pair p+1 starts
            # so they run early (their exp tiles free up the pexp ring).
            xt1 = load_x(1)
            xh0, qt0, kt0, vt0 = [], [], [], []
            gn_batch(0, xt0, xh0)
            # pair-0/1 row-blocks of Q/K emitted directly: each pair's Q/K
            # must be materialized by the time its phase A is emitted.
            for rb in range(2):
                for u in qk_units(0, xh0, qt0, wq_sb, 0, rb):
                    u()
                for u in qk_units(0, xh0, kt0, wk_sb, 4, rb):
                    u()

            xh1, qt1, kt1, vt1 = [], [], [], []
            for st in range(8):
                queue.append(lambda st=st: v_group(0, xh0, vt0, st))
            for rb in range(2, 4):
                queue.extend(qk_units(0, xh0, qt0, wq_sb, 0, rb))
                queue.extend(qk_units(0, xh0, kt0, wk_sb, 4, rb))
            # batch-1 groupnorm deferred into the queue: its PSUM tiles must
            # sit BEHIND batch-0's V/QK in the pps ring, else the in-order PE
            # head-of-line blocks on xt1's DMA through the pool WAR edge.
            queue.append(lambda: gn_batch(1, xt1, xh1))
            for rb in range(2):
                queue.extend(qk_units(1, xh1, qt1, wq_sb, 0, rb))
                queue.extend(qk_units(1, xh1, kt1, wk_sb, 4, rb))
            for st in range(8):
                queue.append(lambda st=st: v_group(1, xh1, vt1, st))
            for rb in range(2, 4):
                queue.extend(qk_units(1, xh1, qt1, wq_sb, 0, rb))
                queue.extend(qk_units(1, xh1, kt1, wk_sb, 4, rb))

            rt0 = [prt.tile([128, S], BF16, tag="rt", name=f"rt0_{i}") for i in range(4)]
            rt1 = [prt.tile([128, S], BF16, tag="rt", name=f"rt1_{i}") for i in range(4)]
            for hp in range(4):
                ex = attn_phaseA(0, qt0, kt0, hp)
                queue[:0] = pb_units(0, vt0, rt0, hp, ex)
            for cb in range(4):
                queue.extend(epi_units(0, xt0, rt0, cb))
            for hp in range(4):
                ex = attn_phaseA(1, qt1, kt1, hp)
                queue[:0] = pb_units(1, vt1, rt1, hp, ex)
            fill(len(queue))
            for cb in range(4):
                for u in epi_units(1, xt1, rt1, cb):
                    u()

    nc.finalize()
    return nc


def kernel(**inputs):
    x = np.asarray(inputs["x"], np.float32)
    norm_w = np.asarray(inputs["norm_w"], np.float64)
    norm_b = np.asarray(inputs["norm_b"], np.float64)
    proj_w = np.asarray(inputs["proj_w"], np.float64)
    proj_b = np.asarray(inputs["proj_b"], np.float64)
    out_w = np.asarray(inputs["out_w"], np.float32)
    out_b = np.asarray(inputs["out_b"], np.float32)

    # split qkv rows (row = h*192 + t*64 + d, t in {q,k,v}) into head-major mats
    pw = proj_w.reshape(NH, 3, DK, C)
    pb = proj_b.reshape(NH, 3, DK)
    mats, biases = [], []
    for t in range(3):
        wm = pw[:, t].reshape(NH * DK, C)
        bv = pb[:, t].reshape(NH * DK)
        # fold groupnorm affine: y = xhat*nw + nb  =>  W@y + b = (W*nw)@xhat + (W@nb + b)
        mats.append(wm * norm_w[None, :])
        biases.append(bv + wm @ norm_b)
    wq, wk, wv = mats
    bq, bk, bv = biases
    scale = DK ** -0.5
    wq = wq * scale
    bq = bq * scale

    wqT = np.ascontiguousarray(wq.T).astype(NPBF16)
    wkT = np.ascontiguousarray(wk.T).astype(NPBF16)
    wvT = np.ascontiguousarray(wv.T).astype(NPBF16)
    woT = np.ascontiguousarray(out_w.T).astype(NPBF16)

    G = np.zeros((128, 8), np.float32)
    G[np.arange(128), np.arange(128) // GSZ] = 1.0
    GT = np.ascontiguousarray(G.T)

    has_bqk = bool(np.any(bq) or np.any(bk))
    has_bv = bool(np.any(bv))
    has_outb = bool(np.any(out_b))

    bqk = np.zeros((128, 8), np.float32)
    bqk[:, 0:4] = bq.reshape(4, 128).T
    bqk[:, 4:8] = bk.reshape(4, 128).T
    outb128 = np.ascontiguousarray(out_b.reshape(4, 128).T)

    nc = _build(has_bqk, has_bv, has_outb)

    xr = x.reshape(B, C, S)
    in_maps = []
    for c in range(N_CORES):
        m = {
            "x": np.ascontiguousarray(xr[c * BL : (c + 1) * BL]),
            "wqt": wqT,
            "wkt": wkT,
            "wvt": wvT,
            "wot": woT,
            "gmat": G,
            "gtmat": GT,
        }
        if has_bqk:
            m["bqk"] = bqk
        if has_bv:
            m["bv"] = np.ascontiguousarray(bv.reshape(1, C)).astype(NPBF16)
        if has_outb:
            m["outb"] = outb128
        in_maps.append(m)

    # guard: bass_utils imports antenv.axon_hooks when tracing is requested
    # (e.g. via BASS_TRACE env); provide a no-op module if the image lacks it.
    try:
        import antenv.axon_hooks  # noqa: F401
    except ImportError:
        import sys
        import types

        import antenv

        _m = types.ModuleType("antenv.axon_hooks")
        _m._hook = None
        _m.set_axon_ntff_profile_hook = lambda h: setattr(_m, "_hook", h)
        _m.get_axon_ntff_profile_hook = lambda: _m._hook
        sys.modules["antenv.axon_hooks"] = _m
        antenv.axon_hooks = _m

    res = None
    for attempt in range(3):
        try:
            res = run_bass_kernel_spmd(
                nc, in_maps, core_ids=list(range(N_CORES)), trace=TRACE
            )
            break
        except Exception:
            # transient NRT_EXEC_UNIT_UNRECOVERABLE-style device hiccups
            # clear on retry; re-raise on the final attempt
            if attempt == 2:
                raise
    LAST["exec_time_ns"] = res.exec_time_ns
    LAST["mean_exec_time_ns"] = res.mean_exec_time_ns
    LAST["result"] = res

    out = np.concatenate([res.results[c]["out"] for c in range(N_CORES)], axis=0)
    return np.ascontiguousarray(out.reshape(B, C, H, W).astype(np.float32))
